# revision 28
# baseline (speedup 1.0000x reference)
"""Trainium2 Bass kernel for nn_Attention_72670846649042.

GRU encoder + greedy attention decoder, B=512,L=25,H=1024,D=256,T=128,E=300.
Sharding: data-parallel over batch, 64 rows/core on 8 cores, no collectives.

v5 design (v3 baseline 1.03 ms):
 - bf16 h-state; bf16 single-pass PE transposes (no fp32 LOW_HIGH); no dummy
   warm transposes.
 - encC via fp8 DoubleRow (M=128, base-0 dst: the only ISA-legal DR shape
   here; DR with col-group dst at partition 64 is invalid ISA).
 - Bias matmuls front-loaded as full-bank K=2 selector matmuls starting each
   PSUM accumulation group (off the critical chain); attn_b folded into EA.
 - Decoder loop reordered: gh + scoresA before the argmax-dependent ohT
   transpose so the in-order PE queue never stalls on the argmax chain.
 - relu -> transpose -> gi pipelined per 256-col half.
 - Log-softmax tail: one Ln + one broadcast subtract + one contiguous DMA.
 - Encoder-critical DMAs issued first; table preloads via dummy activations.
"""
import os
import numpy as np
import ml_dtypes

B, L, V, E, H, D, T = 512, 25, 50000, 300, 1024, 256, 128
NC = 8
BL = B // NC          # 64 local batch
G3 = 3 * H            # 3072
KH = H // 128         # 8 hidden ktiles
MT = 13               # l-pair tiles for attention (25 -> 13 pairs, last padded)
MQ = 7                # l-quad tiles for encC DR layout (28 padded)
MAXN1, MAXN2, BN_EPS = 10.0, 1.0, 1e-5
EK = (128, 128, 45)   # xT/encWih ktile rows (300 rows + 1 ones row)
KORD = (0, 4, 1, 5, 2, 6, 3, 7)   # ktile order gated by chunk-half readiness
BF16 = ml_dtypes.bfloat16
FP8 = ml_dtypes.float8_e4m3

LINEARIZE = False


def build_nc():
    import concourse.bass as bass
    import concourse.tile as tile
    from concourse import bacc, mybir
    from contextlib import ExitStack

    dt = mybir.dt
    AF = mybir.ActivationFunctionType
    ALU = mybir.AluOpType
    AX = mybir.AxisListType
    DRm = mybir.MatmulPerfMode.DoubleRow

    nc = bacc.Bacc("TRN2", target_bir_lowering=False, debug=False)

    # ---- dram parameters (order here = rough DMA priority) ----
    identb_d = nc.declare_dram_parameter("identb", [128, 128], dt.bfloat16, isOutput=False)
    sel2_d = nc.declare_dram_parameter("sel2", [2, 128], dt.bfloat16, isOutput=False)
    xTb_d = nc.declare_dram_parameter("xTb", [301, L * BL], dt.bfloat16, isOutput=False)
    encWihT_d = nc.declare_dram_parameter("encWihT", [301, G3], dt.bfloat16, isOutput=False)
    encWhhT_d = nc.declare_dram_parameter("encWhhT", [H, G3], dt.bfloat16, isOutput=False)
    ebhn_d = nc.declare_dram_parameter("ebhn", [2, 512], dt.bfloat16, isOutput=False)
    decWhhT_d = nc.declare_dram_parameter("decWhhT", [H, G3], dt.bfloat16, isOutput=False)
    decWihT_d = nc.declare_dram_parameter("decWihT", [H, G3], dt.bfloat16, isOutput=False)
    cWapp8_d = nc.declare_dram_parameter("cWapp8", [128, 4 * 2 * H], dt.float8e4, isOutput=False)
    outWTs_d = nc.declare_dram_parameter("outWTs", [H, T], dt.bfloat16, isOutput=False)
    attnWT_d = nc.declare_dram_parameter("attnWT", [H, L], dt.bfloat16, isOutput=False)
    EA_d = nc.declare_dram_parameter("EA", [128, L], dt.bfloat16, isOutput=False)
    EC_d = nc.declare_dram_parameter("EC", [128, H], dt.bfloat16, isOutput=False)
    attnb0_d = nc.declare_dram_parameter("attnb0", [1, L], dt.bfloat16, isOutput=False)
    combb0_d = nc.declare_dram_parameter("combb0", [2, 512], dt.bfloat16, isOutput=False)
    combb_d = nc.declare_dram_parameter("combb", [2, 512], dt.bfloat16, isOutput=False)
    istkb_d = nc.declare_dram_parameter("istkb", [128, MQ * 2 * BL], dt.bfloat16, isOutput=False)
    dgib_d = nc.declare_dram_parameter("dgib", [2, 3 * 512], dt.bfloat16, isOutput=False)
    dbhn_d = nc.declare_dram_parameter("dbhn", [2, 512], dt.bfloat16, isOutput=False)
    lgb_d = nc.declare_dram_parameter("lgb", [1, T], dt.bfloat16, isOutput=False)
    out_d = nc.declare_dram_parameter("out", [BL * L, T], dt.float32, isOutput=True)

    with tile.TileContext(nc, linearize=LINEARIZE) as tc, ExitStack() as ctx:
        shared = ctx.enter_context(tc.tile_pool(name="shared", bufs=1))
        decw = ctx.enter_context(tc.tile_pool(name="decw", bufs=1))

        identb = shared.tile([128, 128], dt.bfloat16, tag="identb")
        nc.sync.dma_start(identb[:], identb_d.ap())
        ones_sb = shared.tile([1, 128], dt.bfloat16, tag="ones_sb")
        nc.vector.memset(ones_sb[:], 1.0)
        sel2 = shared.tile([2, 128], dt.bfloat16, tag="sel2")
        nc.sync.dma_start(sel2[:], sel2_d.ap())

        # table warm-up: sigmoid set loads during initial DMA wait
        dummy = shared.tile([1, 4], dt.float32, tag="dummy")
        nc.vector.memset(dummy[:], 0.0)
        nc.scalar.activation(dummy[:], dummy[:], AF.Sigmoid)

        hA = shared.tile([128, 512], dt.bfloat16, tag="hA")
        hB = shared.tile([128, 512], dt.bfloat16, tag="hB")
        nc.vector.memset(hA[:], 0.0)
        h_tiles = [hA, hB]

        se_all = shared.tile([BL, L], dt.float32, tag="se_all")

        # encoder history: fp8 DR pair layout [p, q, j, l, b] (encC lhsT);
        # hT0 holds the final hidden state for the decoder's first step.
        enc_out8 = decw.tile([128, 4, 2, 26, BL], dt.float8e4, tag="enc_out8")
        nc.vector.memset(enc_out8[:, :, :, 25:26, :], 0.0)
        hT0 = decw.tile([128, KH, BL], dt.bfloat16, tag="hT0")

        def transp_half(hsrc, tp, c):
            # hsrc folded bf16 [128,512]; writes tp[:, f, :, :] for f in
            # {2c, 2c+1}: tp[:, f, hcplane, b] = global ktile (f + 4*hcplane).
            for f in (2 * c, 2 * c + 1):
                nc.tensor.transpose(tp[:, f, :, :], hsrc[:, f * 128:(f + 1) * 128],
                                    identb[:])

        def copyb_half(dst_kslices, tp, c):
            # dst view [128, hc2, f2, b] <- tp[:, 2c:2c+2, :, :] axis-swapped
            nc.vector.tensor_copy(dst_kslices,
                                  tp[:, 2 * c:2 * c + 2, :, :].rearrange(
                                      "p f hc b -> p hc f b"))

        def copy8_half(dst_q_pairs, tp, c):
            # dst view [128, q2(c,c+2), j2, b] <- tp[:, 2c:2c+2, :, :]
            nc.vector.tensor_copy(dst_q_pairs,
                                  tp[:, 2 * c:2 * c + 2, :, :].rearrange(
                                      "p f j b -> p j f b"))

        # =======================================================
        # Phase 1: encoder scan (gi inlined; 25 steps)
        # =======================================================
        with tc.tile_pool(name="encw", bufs=1) as encw, \
             tc.tile_pool(name="egps", bufs=2, space="PSUM") as egps, \
             tc.tile_pool(name="egp1", bufs=1, space="PSUM") as egp1, \
             tc.tile_pool(name="tpp", bufs=1, space="PSUM") as tpp, \
             tc.tile_pool(name="ework", bufs=2) as ework:
            xT = encw.tile([128, 3, L * BL], dt.bfloat16, tag="xT")
            nc.sync.dma_start(xT[:, 0, :], xTb_d.ap()[0:128, :])
            nc.sync.dma_start(xT[:, 1, :], xTb_d.ap()[128:256, :])
            nc.sync.dma_start(xT[0:45, 2, :], xTb_d.ap()[256:301, :])
            eWih = encw.tile([128, 3, G3], dt.bfloat16, tag="eWih")
            nc.sync.dma_start(eWih[:, 0, :], encWihT_d.ap()[0:128, :])
            nc.sync.dma_start(eWih[:, 1, :], encWihT_d.ap()[128:256, :])
            nc.sync.dma_start(eWih[0:45, 2, :], encWihT_d.ap()[256:301, :])
            eWhh = encw.tile([128, KH, G3], dt.bfloat16, tag="eWhh")
            for k in KORD:
                nc.sync.dma_start(
                    eWhh[:, k, :],
                    encWhhT_d.ap().rearrange("(k p) n -> p k n", p=128)[:, k, :])
            ebhn_r = encw.tile([2, 512], dt.bfloat16, tag="ebhn_r")
            nc.sync.dma_start(ebhn_r[:], ebhn_d.ap())
            # bf16 ktile history [p, k, l, b] (encoder gh stationary only)
            enc_outT = encw.tile([128, KH, 25, BL], dt.bfloat16, tag="enc_outT")

            def emit_gi(t, ps_r, ps_z, ps_ngi, rz_stop):
                # groups were started by the bias matmuls in alloc_banks
                for g, bank, stp in ((0, ps_r, rz_stop), (1, ps_z, rz_stop),
                                     (2, ps_ngi, True)):
                    for kt in range(3):
                        for hc in range(2):
                            co = g * H + hc * 512
                            nc.tensor.matmul(
                                bank[hc * 64:(hc + 1) * 64, :],
                                xT[0:EK[kt], kt, t * BL:(t + 1) * BL],
                                eWih[0:EK[kt], kt, co:co + 512],
                                start=(kt == 0), stop=(stp and kt == 2),
                                skip_group_check=True)

            def alloc_banks():
                r = egps.tile([128, 512], dt.float32, name="ps_r", tag="r")
                z = egps.tile([128, 512], dt.float32, name="ps_z", tag="z")
                ngi = egps.tile([128, 512], dt.float32, name="ps_ngi", tag="ngi")
                return r, z, ngi

            banks = {}
            banks[0] = alloc_banks()
            emit_gi(0, *banks[0], rz_stop=True)
            for t in range(L):
                ps_r, ps_z, ps_ngi = banks.pop(t)
                ps_ngh = egp1.tile([128, 512], dt.float32, tag="ngh")
                # ngh bias first (full-bank start, off the critical chain)
                nc.tensor.matmul(ps_ngh[:], sel2[:], ebhn_r[:],
                                 start=True, stop=(t == 0), skip_group_check=True)
                # gh matmuls (skip at t=0: h=0); bank order r, ngh, z so the
                # r/ngh-dependent gate chain starts earliest.
                if t > 0:
                    for g, bank in ((0, ps_r), (2, ps_ngh), (1, ps_z)):
                        for ki, k in enumerate(KORD):
                            for hc in range(2):
                                co = g * H + hc * 512
                                nc.tensor.matmul(
                                    bank[hc * 64:(hc + 1) * 64, :],
                                    enc_outT[:, k, t - 1, :],
                                    eWhh[:, k, co:co + 512],
                                    start=False,
                                    stop=(ki == KH - 1),
                                    skip_group_check=True)
                # next step's gi (fills PE while this step's gate chain runs)
                if t + 1 < L:
                    banks[t + 1] = alloc_banks()
                    emit_gi(t + 1, *banks[t + 1], rz_stop=False)
                # ---- gates, chunked in 256-col halves ----
                hprev = h_tiles[t % 2]
                hnew = h_tiles[(t + 1) % 2]
                r_s = ework.tile([128, 512], dt.bfloat16, tag="r_s")
                z_s = ework.tile([128, 512], dt.bfloat16, tag="z_s")
                nt = ework.tile([128, 512], dt.float32, tag="nt", bufs=1)
                n_s = ework.tile([128, 512], dt.bfloat16, tag="n_s", bufs=1)
                t4 = ework.tile([128, 512], dt.bfloat16, tag="t4", bufs=1)
                tp = tpp.tile([128, 4, 2, BL], dt.bfloat16, tag="tp")
                eo_view = enc_outT[:, :, t, :].rearrange("p (hc f) b -> p hc f b", hc=2)
                for c in range(2):
                    sl = slice(c * 256, (c + 1) * 256)
                    nc.scalar.activation(r_s[:, sl], ps_r[:, sl], AF.Sigmoid)
                    nc.scalar.activation(z_s[:, sl], ps_z[:, sl], AF.Sigmoid)
                    nc.vector.tensor_tensor(nt[:, sl], ps_ngh[:, sl], r_s[:, sl],
                                            op=ALU.mult)
                    nc.vector.tensor_tensor(nt[:, sl], nt[:, sl], ps_ngi[:, sl],
                                            op=ALU.add)
                    nc.scalar.activation(n_s[:, sl], nt[:, sl], AF.Tanh)
                    nc.vector.tensor_tensor(t4[:, sl], hprev[:, sl], n_s[:, sl],
                                            op=ALU.subtract)
                    nc.vector.tensor_tensor(t4[:, sl], t4[:, sl], z_s[:, sl],
                                            op=ALU.mult)
                    nc.vector.tensor_tensor(hnew[:, sl], n_s[:, sl], t4[:, sl],
                                            op=ALU.add)
                    transp_half(hnew, tp, c)
                    copyb_half(eo_view[:, :, 2 * c:2 * c + 2, :], tp, c)
                    copy8_half(enc_out8[:, c:c + 3:2, :, t, :], tp, c)
            nc.vector.tensor_copy(hT0[:], enc_outT[:, :, 24, :])

        # =======================================================
        # Phase 2: encC = enc_out @ combW_app (fp8 DR, M=128 base 0)
        # =======================================================
        decw2 = ctx.enter_context(tc.tile_pool(name="decw2", bufs=1))
        decWhhT = decw2.tile([128, KH, G3], dt.bfloat16, tag="decWhhT")
        nc.sync.dma_start(decWhhT[:], decWhhT_d.ap().rearrange("(k p) n -> p k n", p=128))
        decWihT = decw2.tile([128, KH, G3], dt.bfloat16, tag="decWihT")
        nc.sync.dma_start(decWihT[:], decWihT_d.ap().rearrange("(k p) n -> p k n", p=128))
        outWTs = decw2.tile([128, KH, T], dt.bfloat16, tag="outWTs")
        nc.sync.dma_start(outWTs[:], outWTs_d.ap().rearrange("(k p) n -> p k n", p=128))
        attnWT = decw2.tile([128, KH, L], dt.bfloat16, tag="attnWT")
        nc.sync.dma_start(attnWT[:], attnWT_d.ap().rearrange("(k p) n -> p k n", p=128))
        EA = decw2.tile([128, L], dt.bfloat16, tag="EA")
        nc.sync.dma_start(EA[:], EA_d.ap())
        EC = decw2.tile([128, H], dt.bfloat16, tag="EC")
        nc.sync.dma_start(EC[:], EC_d.ap())
        IstkB = decw2.tile([128, MQ, 2, BL], dt.bfloat16, tag="IstkB")
        nc.sync.dma_start(IstkB[:], istkb_d.ap())
        attnb0_r = decw2.tile([1, L], dt.bfloat16, tag="attnb0_r")
        nc.sync.dma_start(attnb0_r[:], attnb0_d.ap())
        combb0_r = decw2.tile([2, 512], dt.bfloat16, tag="combb0_r")
        nc.sync.dma_start(combb0_r[:], combb0_d.ap())
        combb_r = decw2.tile([2, 512], dt.bfloat16, tag="combb_r")
        nc.sync.dma_start(combb_r[:], combb_d.ap())
        dgib_r = decw2.tile([2, 3 * 512], dt.bfloat16, tag="dgib_r")
        nc.sync.dma_start(dgib_r[:], dgib_d.ap())
        dbhn_r = decw2.tile([2, 512], dt.bfloat16, tag="dbhn_r")
        nc.sync.dma_start(dbhn_r[:], dbhn_d.ap())
        lgb_r = decw2.tile([1, T], dt.bfloat16, tag="lgb_r")
        nc.sync.dma_start(lgb_r[:], lgb_d.ap())
        # encC bf16 in quad-interleaved layout [p, qm, jm, H]; pad pair zeroed
        encCB = decw2.tile([128, MQ, 2, H], dt.bfloat16, tag="encCB")
        nc.vector.memset(encCB[:, MQ - 1, 1, :], 0.0)
        lg_all = decw2.tile([BL, L, T], dt.float32, tag="lg_all")

        # =======================================================
        # Phase 3: decoder (25 steps)
        # =======================================================
        with tc.tile_pool(name="dgps", bufs=1, space="PSUM") as dgps, \
             tc.tile_pool(name="dops", bufs=1, space="PSUM") as dops, \
             tc.tile_pool(name="tpp2", bufs=1, space="PSUM") as tpp2, \
             tc.tile_pool(name="mscp", bufs=1, space="PSUM") as mscp, \
             tc.tile_pool(name="lgps", bufs=1, space="PSUM") as lgps, \
             tc.tile_pool(name="dwork", bufs=1) as dwork:
            hTt = shared.tile([128, KH, BL], dt.bfloat16, tag="hTt")
            oTt = shared.tile([128, KH, BL], dt.bfloat16, tag="oTt")
            awn = shared.tile([128, 2 * MQ], dt.float32, tag="awn")
            nc.vector.memset(awn[0:BL, 13:14], 0.0)
            nc.vector.memset(awn[BL:128, 12:14], 0.0)
            hT_view = hTt[:].rearrange("p (hc f) b -> p hc f b", hc=2)
            oT_view = oTt[:].rearrange("p (hc f) b -> p hc f b", hc=2)
            oh_prev = None

            def emit_front_r(t):
                # r-bank bias + first-half gh-r for step t; emitted inside the
                # previous step's gates window (after hT half-0 is copied).
                hTk = (lambda k: hT0[:, k, :]) if t == 0 else \
                      (lambda k: hTt[:, k, :])
                ps_r = dgps.tile([128, 512], dt.float32, tag="r")
                nc.tensor.matmul(ps_r[:], sel2[:], dgib_r[:, 0:512],
                                 start=True, stop=False, skip_group_check=True)
                for k in (0, 4, 1, 5):
                    for hc in range(2):
                        nc.tensor.matmul(ps_r[hc * 64:(hc + 1) * 64, :],
                                         hTk(k),
                                         decWhhT[:, k, hc * 512:hc * 512 + 512],
                                         start=False, stop=False,
                                         skip_group_check=True)
                return ps_r, hTk

            def emit_front(t, pre):
                ps_r, hTk = pre
                ps_z = dgps.tile([128, 512], dt.float32, tag="z")
                ps_ngh = dgps.tile([128, 512], dt.float32, tag="ngh")
                ps_ngi = dgps.tile([128, 512], dt.float32, tag="ngi")
                ps_o = dops.tile([128, 512], dt.float32, tag="o")
                misc = mscp.tile([128, 512], dt.float32, tag="misc")
                sc = misc[0:BL, 128:128 + L]
                for g, bank in ((1, ps_z), (2, ps_ngi)):
                    nc.tensor.matmul(bank[:], sel2[:], dgib_r[:, g * 512:(g + 1) * 512],
                                     start=True, stop=False, skip_group_check=True)
                nc.tensor.matmul(ps_ngh[:], sel2[:], dbhn_r[:],
                                 start=True, stop=False, skip_group_check=True)
                nc.tensor.matmul(ps_o[:], sel2[:],
                                 combb0_r[:] if t == 0 else combb_r[:],
                                 start=True, stop=False, skip_group_check=True)
                for k in (2, 6, 3, 7):
                    for hc in range(2):
                        nc.tensor.matmul(ps_r[hc * 64:(hc + 1) * 64, :],
                                         hTk(k),
                                         decWhhT[:, k, hc * 512:hc * 512 + 512],
                                         start=False, stop=False,
                                         skip_group_check=True)
                for ki, k in enumerate(KORD):
                    nc.tensor.matmul(sc, hTk(k), attnWT[:, k, :],
                                     start=(ki == 0), stop=False)
                return ps_r, ps_z, ps_ngh, ps_ngi, ps_o, sc, hTk

            # ---- encC inside the decoder pools: per-bank WAR ordering
            # instead of a pool barrier; decoder front(0) fills the copy tail.
            rot = [dgps.tile([128, 512], dt.float32, name=f"cc_{x}", tag=x)
                   for x in ("r", "z", "ngh", "ngi")]
            cc_o = dops.tile([128, 512], dt.float32, tag="o")
            rot.append(cc_o)
            cc_m = mscp.tile([128, 512], dt.float32, tag="misc")
            rot.append(cc_m)
            pre0 = None
            with tc.tile_pool(name="ccw", bufs=1) as ccw:
                cWapp8 = ccw.tile([128, 4, 2, H], dt.float8e4, tag="cWapp8")
                nc.sync.dma_start(cWapp8[:], cWapp8_d.ap())
                for idx in range(2 * MT):
                    m, nch = idx // 2, idx % 2
                    # keep ps_r (rot[0]) free of the last groups so front_r(0)
                    # can start while encC drains
                    bank = rot[idx % 6] if idx < 19 else rot[1 + (idx - 19) % 5]
                    for q in range(4):
                        nc.tensor.matmul(
                            bank[:],
                            enc_out8[:, q, :, 2 * m:2 * m + 2, :].rearrange(
                                "p j l b -> p j (l b)"),
                            cWapp8[:, q, :, nch * 512:(nch + 1) * 512],
                            start=(q == 0), stop=(q == 3), perf_mode=DRm,
                            skip_group_check=True)
                    nc.vector.tensor_copy(
                        encCB[:, m // 2, m % 2, nch * 512:nch * 512 + 320],
                        bank[:, 0:320])
                    nc.scalar.copy(
                        encCB[:, m // 2, m % 2, nch * 512 + 320:(nch + 1) * 512],
                        bank[:, 320:512])
                    if idx == 19:
                        pre0 = emit_front_r(0)
            # decoder exp/tanh table preload (anchored to encoder output so
            # the scheduler cannot hoist it to kernel start)
            nc.scalar.activation(dummy[:], hT0[0:1, 0, 0:4], AF.Exp)
            front = emit_front(0, pre0)
            for t in range(L):
                ps_r, ps_z, ps_ngh, ps_ngi, ps_o, sc, hTk = front
                lg = lgps.tile([BL, T], dt.float32, tag="lg")
                # ---- ohT from prev argmax, then scores part B ----
                if t > 0:
                    tp0 = tpp2.tile([128, 4, 2, BL], dt.bfloat16, tag="tp")
                    nc.tensor.transpose(tp0[:, 0, 0, :], oh_prev[:],
                                        identb[0:BL, 0:BL])
                    ohT = dwork.tile([128, BL], dt.bfloat16, tag="ohT")
                    nc.vector.tensor_copy(ohT[:], tp0[:, 0, 0, :])
                    nc.tensor.matmul(sc, ohT[:], EA[:], start=False, stop=True)
                else:
                    nc.tensor.matmul(sc, ones_sb[0:1, 0:BL], attnb0_r[:],
                                     start=False, stop=True)
                # ---- EC part of o ----
                if t > 0:
                    for hc in range(2):
                        nc.tensor.matmul(ps_o[hc * 64:(hc + 1) * 64, :], ohT[:],
                                         EC[:, hc * 512:(hc + 1) * 512],
                                         start=False, stop=False,
                                         skip_group_check=True)
                # ---- gh ngh-bank ----
                for ki, k in enumerate(KORD):
                    for hc in range(2):
                        co = 2 * H + hc * 512
                        nc.tensor.matmul(ps_ngh[hc * 64:(hc + 1) * 64, :],
                                         hTk(k), decWhhT[:, k, co:co + 512],
                                         start=False, stop=(ki == KH - 1),
                                         skip_group_check=True)
                # ---- softmax chain (ACT/DVE, overlaps the gh matmuls) ----
                aw = dwork.tile([BL, L], dt.float32, tag="aw")
                nc.scalar.activation(aw[:], sc, AF.Exp)
                sume = dwork.tile([BL, 1], dt.float32, tag="sume")
                nc.vector.tensor_reduce(sume[:], aw[:], axis=AX.X, op=ALU.add)
                rs = dwork.tile([BL, 1], dt.float32, tag="rs")
                nc.vector.reciprocal(rs[:], sume[:])
                rs2 = dwork.tile([128, 1], dt.float32, tag="rs2")
                nc.vector.tensor_copy(rs2[0:BL, :], rs[:])
                nc.vector.tensor_copy(rs2[BL:128, :], rs[:])
                # awn[(qm jm)] bf16: even l on partitions 0:64, odd on 64:128
                nc.vector.tensor_copy(awn[0:BL, 0:13], aw[:, 0:25:2])
                nc.vector.tensor_copy(awn[BL:128, 0:12], aw[:, 1:25:2])
                nc.vector.tensor_scalar(awn[:], awn[:], rs2[:], None, op0=ALU.mult)
                dgs = dwork.tile([128, MQ, 2, BL], dt.bfloat16, tag="dgs", bufs=1)
                for qa, qb in ((0, 4), (4, MQ)):
                    nc.vector.tensor_tensor(
                        dgs[:, qa:qb, :, :], IstkB[:, qa:qb, :, :],
                        awn[:, 2 * qa:2 * qb].rearrange("p (q j) -> p q j", j=2)
                        .broadcast_to((128, qb - qa, 2, BL)),
                        op=ALU.mult)
                # ---- einsum in two halves (first half starts sooner) ----
                for p in range(2 * MQ - 1):
                    for hc in range(2):
                        nc.tensor.matmul(ps_o[hc * 64:(hc + 1) * 64, :],
                                         dgs[:, p // 2, p % 2, :],
                                         encCB[:, p // 2, p % 2,
                                               hc * 512:(hc + 1) * 512],
                                         start=False,
                                         stop=(p == 2 * MQ - 2),
                                         skip_group_check=True)
                # ---- gh z-bank (covers the relu/transpose window) ----
                for ki, k in enumerate(KORD):
                    for hc in range(2):
                        co = H + hc * 512
                        nc.tensor.matmul(ps_z[hc * 64:(hc + 1) * 64, :],
                                         hTk(k), decWhhT[:, k, co:co + 512],
                                         start=False, stop=False,
                                         skip_group_check=True)
                obf = dwork.tile([128, 512], dt.bfloat16, tag="obf")
                # ---- relu -> oT -> gi, pipelined per 256-col half ----
                tp = tpp2.tile([128, 4, 2, BL], dt.bfloat16, tag="tp")
                for c in range(2):
                    sl = slice(c * 256, (c + 1) * 256)
                    nc.scalar.activation(obf[:, sl], ps_o[:, sl], AF.Relu,
                                         scale=S2_SCALE)
                    transp_half(obf, tp, c)
                    copyb_half(oT_view[:, :, 2 * c:2 * c + 2, :], tp, c)
                    for k in (2 * c, 2 * c + 4, 2 * c + 1, 2 * c + 5):
                        for hc in range(2):
                            nc.tensor.matmul(ps_r[hc * 64:(hc + 1) * 64, :],
                                             oTt[:, k, :],
                                             decWihT[:, k, hc * 512:hc * 512 + 512],
                                             start=False,
                                             stop=(c == 1 and k == 2 * c + 5),
                                             skip_group_check=True)
                for g, bank in ((2, ps_ngi), (1, ps_z)):
                    for ki, k in enumerate(KORD):
                        for hc in range(2):
                            co = g * H + hc * 512
                            nc.tensor.matmul(bank[hc * 64:(hc + 1) * 64, :],
                                             oTt[:, k, :],
                                             decWihT[:, k, co:co + 512],
                                             start=False,
                                             stop=(ki == KH - 1),
                                             skip_group_check=True)
                # logits bias (must precede the first logits matmul below)
                nc.tensor.matmul(lg[:], ones_sb[0:1, 0:BL], lgb_r[:],
                                 start=True, stop=False)
                # ---- gates (sigma via tanh), chunked halves ----
                hprev = h_tiles[(L + t) % 2]
                hnew = h_tiles[(L + t + 1) % 2]
                r_s = dwork.tile([128, 512], dt.bfloat16, tag="r_s")
                z_s = dwork.tile([128, 512], dt.bfloat16, tag="z_s")
                nt = dwork.tile([128, 512], dt.float32, tag="nt", bufs=1)
                n_s = dwork.tile([128, 512], dt.bfloat16, tag="n_s", bufs=1)
                t4 = dwork.tile([128, 512], dt.bfloat16, tag="t4", bufs=1)
                tp2 = tpp2.tile([128, 4, 2, BL], dt.bfloat16, tag="tp")
                for c in range(2):
                    sl = slice(c * 256, (c + 1) * 256)
                    nc.scalar.activation(r_s[:, sl], ps_r[:, sl], AF.Tanh, scale=0.5)
                    nc.vector.tensor_scalar(r_s[:, sl], r_s[:, sl], 0.5, 0.5,
                                            op0=ALU.mult, op1=ALU.add)
                    nc.scalar.activation(z_s[:, sl], ps_z[:, sl], AF.Tanh, scale=0.5)
                    nc.vector.tensor_scalar(z_s[:, sl], z_s[:, sl], 0.5, 0.5,
                                            op0=ALU.mult, op1=ALU.add)
                    nc.vector.tensor_tensor(nt[:, sl], ps_ngh[:, sl], r_s[:, sl],
                                            op=ALU.mult)
                    nc.vector.tensor_tensor(nt[:, sl], nt[:, sl], ps_ngi[:, sl],
                                            op=ALU.add)
                    nc.scalar.activation(n_s[:, sl], nt[:, sl], AF.Tanh)
                    nc.vector.tensor_tensor(t4[:, sl], hprev[:, sl], n_s[:, sl],
                                            op=ALU.subtract)
                    nc.vector.tensor_tensor(t4[:, sl], t4[:, sl], z_s[:, sl],
                                            op=ALU.mult)
                    nc.vector.tensor_tensor(hnew[:, sl], n_s[:, sl], t4[:, sl],
                                            op=ALU.add)
                    transp_half(hnew, tp2, c)
                    copyb_half(hT_view[:, :, 2 * c:2 * c + 2, :], tp2, c)
                    # logits for the k-tiles this half provides
                    for ki, k in enumerate((2 * c, 2 * c + 4, 2 * c + 1, 2 * c + 5)):
                        nc.tensor.matmul(lg[:], hTt[:, k, :], outWTs[:, k, :],
                                         start=False,
                                         stop=(c == 1 and ki == 3))
                    if c == 0 and t + 1 < L:
                        pre = emit_front_r(t + 1)
                if t + 1 < L:
                    front = emit_front(t + 1, pre)
                ex = dwork.tile([BL, T], dt.float32, tag="ex")
                nc.scalar.activation(ex[:], lg[:], AF.Exp,
                                     accum_out=se_all[:, t:t + 1])
                nc.scalar.copy(lg_all[:, t, :], lg[:])
                # ---- argmax onehot (transposed next iteration) ----
                if t < L - 1:
                    mx2 = dwork.tile([BL, 1], dt.float32, tag="mx2")
                    nc.vector.tensor_reduce(mx2[:], lg[:], axis=AX.X, op=ALU.max)
                    oh_prev = dwork.tile([BL, T], dt.bfloat16, tag="oh")
                    nc.vector.tensor_scalar(oh_prev[:], lg[:], mx2[:], None,
                                            op0=ALU.is_equal)

        # =======================================================
        # Phase 4: log-softmax tail (one Ln, one subtract, one DMA)
        # =======================================================
        with tc.tile_pool(name="tail", bufs=1) as tail:
            lse = tail.tile([BL, L], dt.float32, tag="lse")
            nc.scalar.activation(lse[:], se_all[:], AF.Ln)
            for lo, hi in ((0, 13), (13, L)):
                nc.vector.tensor_tensor(
                    lg_all[:, lo:hi, :],
                    lg_all[:, lo:hi, :],
                    lse[:, lo:hi].rearrange("b (l o) -> b l o", o=1)
                    .broadcast_to((BL, hi - lo, T)),
                    op=ALU.subtract)
                nc.sync.dma_start(
                    out_d.ap().rearrange("(b l) c -> b l c", l=L)[:, lo:hi, :]
                    .rearrange("b l c -> b (l c)"),
                    lg_all[:, lo:hi, :].rearrange("b l c -> b (l c)"))
    nc.finalize()
    return nc


S2_SCALE = 1.0  # patched at build time (bn2 scale); module-level for closure use


def kernel(**inputs):
    global S2_SCALE
    import concourse.bass_utils as bass_utils

    tokens = np.asarray(inputs["tokens"])
    w2v = np.asarray(inputs["w2v"], np.float32)
    bn1 = np.asarray(inputs["bn1"], np.float32)
    bn2 = np.asarray(inputs["bn2"], np.float32)
    s1 = float(bn1[0] / np.sqrt(bn1[3] + BN_EPS))
    t1 = float(bn1[1] - bn1[2] * s1)
    s2 = float(bn2[0] / np.sqrt(bn2[3] + BN_EPS))
    t2 = float(bn2[1] - bn2[2] * s2)
    S2_SCALE = s2

    f32 = lambda k: np.asarray(inputs[k], np.float32)
    bft = lambda a: np.ascontiguousarray(np.asarray(a, np.float32).T).astype(BF16)

    def dr_pack(WT, n):
        # WT [1024, n] fp32 -> [128, 4*2*n] fp8: (p, (q, j, col)) =
        # WT[(2q+j)*128 + p, col]
        return np.ascontiguousarray(
            WT.reshape(4, 2, 128, n).transpose(2, 0, 1, 3)).astype(FP8).reshape(128, -1)

    enc_bih, enc_bhh = f32("enc_bih"), f32("enc_bhh")
    dec_bih, dec_bhh = f32("dec_bih"), f32("dec_bhh")
    egib = np.concatenate([enc_bih[:H] + enc_bhh[:H], enc_bih[H:2 * H] + enc_bhh[H:2 * H],
                           enc_bih[2 * H:]])
    dgib = np.concatenate([dec_bih[:H] + dec_bhh[:H], dec_bih[H:2 * H] + dec_bhh[H:2 * H],
                           dec_bih[2 * H:]])
    out_W = f32("out_W")
    lgb = (f32("out_b") + t1 * out_W.sum(axis=1))[None, :]
    combb = (f32("comb_b") + t2 / s2)[None, :]
    comb_W = f32("comb_W")
    attn_W = f32("attn_W")
    attnb = f32("attn_b")[None, :]

    # encoder Wih with gi bias folded as last row
    encWihT = np.zeros((301, G3), np.float32)
    encWihT[:300] = f32("enc_Wih").T
    encWihT[300] = egib

    # dec_emb rows 0..127 renormed; fold emb@attnW_emb + attn_b into EA
    em = f32("dec_emb")[:T]
    emn = np.linalg.norm(em, axis=1, keepdims=True)
    embf = em * np.minimum(1.0, MAXN2 / (emn + 1e-7))
    EA = embf @ attn_W[:, :D].T + attnb               # (128, L)
    EC = embf @ comb_W[:, :D].T                       # (128, H)
    sos = f32("dec_emb")[T]
    sos = sos * min(1.0, MAXN2 / (np.linalg.norm(sos) + 1e-7))
    attnb0 = attnb + (sos @ attn_W[:, :D].T)[None, :]
    combb0 = combb + (sos @ comb_W[:, :D].T)[None, :]

    identb = np.eye(128, dtype=np.float32).astype(BF16)
    sel2 = np.zeros((2, 128), np.float32)
    sel2[0, 0:64] = 1.0
    sel2[1, 64:128] = 1.0
    # istk in quad layout [128, MQ, 2, BL]; out-of-range l lines zeroed
    istk = np.zeros((128, BL), np.float32)
    istk[np.arange(128), np.arange(128) % BL] = 1.0
    istkb = np.tile(istk[:, None, None, :], (1, MQ, 2, 1))
    for m in range(2 * MQ):      # pair m covers l = (2m, 2m+1)
        q, j = m // 2, m % 2
        if 2 * m >= L:
            istkb[:64, q, j, :] = 0.0
        if 2 * m + 1 >= L:
            istkb[64:, q, j, :] = 0.0

    common = {
        "identb": identb,
        "sel2": sel2.astype(BF16),
        "encWihT": encWihT.astype(BF16),
        "encWhhT": bft(inputs["enc_Whh"]),
        "decWihT": bft(inputs["dec_Wih"]),
        "decWhhT": bft(inputs["dec_Whh"]),
        "cWapp8": dr_pack(np.ascontiguousarray(comb_W[:, D:].T), H),
        "outWTs": np.ascontiguousarray((s1 * out_W).T).astype(BF16),
        "attnWT": np.ascontiguousarray(attn_W[:, D:].T).astype(BF16),
        "EA": np.ascontiguousarray(EA).astype(BF16),
        "EC": np.ascontiguousarray(EC).astype(BF16),
        "attnb0": np.ascontiguousarray(attnb0).astype(BF16),
        "combb0": np.ascontiguousarray(combb0.reshape(2, 512)).astype(BF16),
        "combb": np.ascontiguousarray(combb.reshape(2, 512)).astype(BF16),
        "istkb": istkb.reshape(128, MQ * 2 * BL).astype(BF16),
        "ebhn": np.ascontiguousarray(enc_bhh[2 * H:].reshape(2, 512)).astype(BF16),
        "dgib": np.ascontiguousarray(
            dgib.reshape(3, 2, 512).transpose(1, 0, 2).reshape(2, -1)).astype(BF16),
        "dbhn": np.ascontiguousarray(dec_bhh[2 * H:].reshape(2, 512)).astype(BF16),
        "lgb": np.ascontiguousarray(lgb).astype(BF16),
    }
    in_maps = []
    for c in range(NC):
        tok = tokens[c * BL:(c + 1) * BL].astype(np.int64)        # (64,25)
        xg = w2v[tok]                                             # (64,25,300)
        nrm = np.linalg.norm(xg, axis=-1, keepdims=True)
        xg = xg * np.minimum(1.0, MAXN1 / (nrm + 1e-7))
        xTb = np.zeros((301, L * BL), np.float32)
        xTb[:300] = xg.transpose(2, 1, 0).reshape(E, L * BL)      # col = l*64+b
        xTb[300] = 1.0
        m = dict(common)
        m["xTb"] = xTb.astype(BF16)
        in_maps.append(m)

    nc = build_nc()
    trace = bool(int(os.environ.get("KERNEL_TRACE", "0")))
    res = bass_utils.run_bass_kernel_spmd(nc, in_maps, core_ids=list(range(NC)),
                                          trace=trace)
    if trace and res.exec_time_ns is not None:
        print(f"HW exec time: {res.exec_time_ns} ns", flush=True)
        print("trace:", res.instructions_and_trace[1] if res.instructions_and_trace else None,
              flush=True)
    out = np.concatenate([res.results[c]["out"] for c in range(NC)], axis=0)
    return out.astype(np.float32)


if __name__ == "__main__":
    pass
